# revision 7
# baseline (speedup 1.0000x reference)
"""TransE-style GNN message-passing scoring kernel for 8 Trainium2 NeuronCores.

Math: reference computes scores[r,e] = sum_d(ent[src]+rel[rl]-ent[dst])[d].
The sum over d is linear, so scores = S[src] + R[rl] - S[dst] where
S = rowsum(ent_table) [1M], R = rowsum(rel_table) [1000].

V2 design (per core c = relation row c):
  phase 1: stream 1/8 of ent_table in [128,32,128] super-row tiles (16KB
           contiguous per partition), rowsum via a TT-add tree on DVE
           (2-operand TTs avoid the DVE 2-port perf modes that lock the Q7
           SWDGE cores out of SBUF and stall gather descriptor generation).
           Tree output lands directly in bf16.
  phase 1b: replicate each 32-entry block x4 in SBUF (block-replicated
           bf16 table rows of 256B), one natural-order DMA to DRAM,
           AllGather -> Tg[31744 rows x 128 bf16] = 8.1MB shared table.
           hi = idx>>5 fits int16 (31744 rows).  rel_table rowsums get the
           same 32x128 bf16-x4 local table (8KB, no collective).
  phase 2: dma_gather 256B granules (the SWDGE floor) from Tg / reltab on
           4 SWDGE queues (the ucode max; ~8ns/idx/queue emission is the
           kernel's wall).  rel gathers are issued first so they drain
           under phase 1.  Select lane lo=idx&31 from the 32 bf16
           candidates with an iota/is_equal one-hot, multiply, and a TT-add
           tree (again 2-operand to keep the Q7s unblocked).
  phase 3: score = sel(src) + sel(rel) - sel(dst); 32x32 stream-transposes
           + block-permuted DMA writes out[131072] in edge order; zero tail.
Host does only integer index prep (hi/lo split + SWDGE wrap layout) and
tensor sharding/concat; all FP math runs on device.
"""

import numpy as np

N_ENT = 1_000_000
DIM = 128
R_TYPES = 8
E_PER_TYPE = 131_072
SCORE_DIM = 150_000
N_REL = 1_000
N_CORES = 8

FULL_CFG = dict(
    tiles=992,          # ent rows/128 per core (992*128 = 126,976)
    batch=32,           # stream rows per partition per tile
    e_cols=E_PER_TYPE // 128,   # 1024 score slots ([128, e_cols] per core)
    jch=8192,           # indices per dma_gather instruction
    score_dim=SCORE_DIM,
    queues=4,
)

SHIFT = 5               # table row = idx >> 5 (32 entities per 256B row)
MASK = 31


def build_nc(cfg):
    import concourse.bass as bass
    import concourse.bacc as bacc
    import concourse.tile as tile
    from concourse import mybir

    f32 = mybir.dt.float32
    bf16 = mybir.dt.bfloat16
    i16 = mybir.dt.int16
    AX = mybir.AxisListType
    OP = mybir.AluOpType

    TILES = cfg["tiles"]
    B = cfg["batch"]
    S = cfg["e_cols"]              # score slots (free dim of [128, S])
    JCH = cfg["jch"]
    SCORE = cfg["score_dim"]
    NQ = cfg["queues"]
    ROWS = TILES * 128             # 126,976 ent rows per core
    NB = TILES // B                # stream tiles (31)
    NEDGE = 128 * S                # 131,072 edges per core
    HCOLS = NEDGE // 16            # wrapped-idx columns (8192)
    NCH = NEDGE // JCH             # gather chunks per stream (16)
    SC = JCH // 128                # score slots per chunk (64)
    CHW = JCH // 16                # hi columns per chunk (512)
    NG2 = S // 32                  # 32-col writeout groups (32)
    LROWS = ROWS // 32             # local table rows (3968)
    AGL = LROWS * 128              # local replicated table elems (507,904)
    SGL = N_CORES * AGL            # global table elems (4,063,232)
    GROWS = SGL // 128             # global table rows (31,744)
    assert TILES % B == 0 and NEDGE % JCH == 0 and JCH % 128 == 0
    assert GROWS < 32768
    PAD = SCORE - NEDGE
    assert PAD % 16 == 0

    nc = bacc.Bacc(None, num_devices=N_CORES, num_swdge_queues=NQ)
    ent = nc.dram_tensor("ent_shard", [ROWS, DIM], f32, kind="ExternalInput")
    rel = nc.dram_tensor("rel_table", [N_REL, DIM], f32, kind="ExternalInput")
    his = {}
    los = {}
    for st in ("src", "dst", "rel"):
        his[st] = nc.dram_tensor(f"hi_{st}", [128, HCOLS], i16, kind="ExternalInput")
        los[st] = nc.dram_tensor(f"lo_{st}", [128, S], f32, kind="ExternalInput")
    out = nc.dram_tensor("out", [SCORE], f32, kind="ExternalOutput")
    iota = nc.inline_tensor(
        np.tile(np.arange(32, dtype=np.float32), (128, 1)), name="iota32")
    wtab = nc.inline_tensor(np.zeros((1, 128), dtype=np.float32), name="warmtab")
    widx = nc.inline_tensor(np.zeros((128, 8), dtype=np.int16), name="warmidx")

    with tile.TileContext(nc) as tc:
        with tc.tile_pool(name="stream", bufs=2) as p_st, \
             tc.tile_pool(name="persist", bufs=1) as p_p, \
             tc.tile_pool(name="scratch", bufs=2) as p_scr, \
             tc.tile_pool(name="hi", bufs=4) as p_hi, \
             tc.tile_pool(name="gout", bufs=3) as p_go, \
             tc.tile_pool(name="mask", bufs=2) as p_mk, \
             tc.tile_pool(name="tree", bufs=2) as p_tr, \
             tc.tile_pool(name="dram", bufs=1, space="DRAM") as p_d:

            ag_in = p_d.tile([AGL], bf16)
            sg = p_d.tile([SGL], bf16, addr_space="Shared")
            rg = p_d.tile([1024], f32)
            reltab = p_d.tile([4096], bf16)

            # ---- warm-up gathers: pull the Q7 IRAM library load to t~0 on
            # all 4 SWDGE queues (each queue = its own pair of Q7 cores) ----
            wi_sb = p_p.tile([128, 8], i16)
            nc.sync.dma_start(wi_sb[:], widx[:])
            for q in range(NQ):
                wg = p_p.tile([128, 1, 128], f32, name=f"wg{q}")
                nc.gpsimd.dma_gather(wg[:], wtab[:], wi_sb[:], 128, 128, 128,
                                     single_packet=False, queue_num=q)

            # ---- zero-pad out tail early (independent of all phases) ----
            z = p_p.tile([16, PAD // 16], f32)
            nc.vector.memset(z[:], 0.0)
            nc.sync.dma_start(out[NEDGE:SCORE].rearrange("(p f) -> p f", p=16), z[:])

            # ---- rel-table chain on the scalar HWDGE ring so the sync ring
            # can start streaming ent immediately ----
            r_in = p_scr.tile([125, 8, 128], f32, tag="scr", name="r_in")
            nc.scalar.dma_start(r_in[:], rel[0:1000, :].rearrange("(p b) d -> p b d", b=8))
            rs = p_p.tile([128, 8], f32)
            nc.vector.memset(rs[:], 0.0)
            nc.vector.tensor_reduce(rs[:125, :], r_in[:], axis=AX.X, op=OP.add)
            nc.scalar.dma_start(rg[:].rearrange("(p b) -> p b", b=8), rs[:])
            rel_nat = p_p.tile([32, 32], f32)
            nc.scalar.dma_start(rel_nat[:], rg[:].rearrange("(p c) -> p c", c=32))
            rel_rep = p_p.tile([32, 4, 32], bf16)
            nc.vector.tensor_copy(rel_rep[:],
                                  rel_nat[:].rearrange("p (o c) -> p o c", o=1)
                                  .to_broadcast([32, 4, 32]))
            nc.scalar.dma_start(reltab[:].rearrange("(p f) -> p f", f=128), rel_rep[:])
            relv = reltab[:].rearrange("(n e) -> n e", e=128)    # [32, 128]

            # ---- index metadata loads (scalar ring, needed ~30us in);
            # hi slices stream per-chunk from a small pool in sel_chunk ----
            lo_bf = {}
            for st in ("rel", "src", "dst"):
                lo_f = p_scr.tile([128, S], f32, tag="lo", name=f"lo_{st}_f")
                nc.scalar.dma_start(lo_f[:], los[st][:])
                lo_bf[st] = p_p.tile([128, S], bf16, name=f"lo_{st}_bf")
                nc.vector.tensor_copy(lo_bf[st][:], lo_f[:])
            iota_f = p_p.tile([128, 32], f32)
            nc.sync.dma_start(iota_f[:], iota[:])
            iota_bf = p_p.tile([128, 32], bf16)
            nc.vector.tensor_copy(iota_bf[:], iota_f[:])

            score = p_p.tile([128, S], f32)
            rel_val = p_p.tile([128, S], f32)
            tr2 = p_p.tile([128, S], f32)
            sgv = sg[:].rearrange("(n e) -> n e", e=128)         # [31744, 128]
            ov = out[0:NEDGE].rearrange("(g i a j) -> a i g j", g=NG2, i=32, a=4, j=32)
            qctr = [0]

            def tt_tree(src_ap, width, dst_ap, pool, tag):
                """Reduce innermost `width` lanes by repeated halving TT-adds.

                src_ap: [128, n, width] bf16; dst_ap: [128, n(,1)] f32.
                2-operand TTs only (never triggers DVE 2-port perf modes).
                """
                cur = src_ap
                w = width
                while w > 2:
                    h = w // 2
                    nxt = pool.tile([128, cur.shape[1], h], bf16, tag=f"{tag}{h}",
                                    name=f"{tag}{h}")
                    nc.vector.tensor_tensor(nxt[:], cur[:, :, 0:h], cur[:, :, h:w],
                                            op=OP.add)
                    cur = nxt
                    w = h
                nc.vector.tensor_tensor(dst_ap, cur[:, :, 0:1], cur[:, :, 1:2],
                                        op=OP.add)

            def sel_chunk(st, table_v, k, dst_ap):
                """Gather chunk k of stream st, one-hot select lane lo, write
                [128, SC] f32 into dst_ap."""
                hi_t = p_hi.tile([128, CHW], i16, tag="hi", name=f"hi_{st}{k}")
                nc.scalar.dma_start(hi_t[:], his[st][:, CHW * k:CHW * (k + 1)])
                gout = p_go.tile([128, SC, 128], bf16, tag="gout", name="gout")
                nc.gpsimd.dma_gather(
                    gout[:], table_v, hi_t[:], JCH, JCH, 128,
                    single_packet=False, queue_num=qctr[0] % NQ)
                qctr[0] += 1
                mk = p_mk.tile([128, SC, 32], bf16, tag="mk", name="mk")
                lo_b = (lo_bf[st][:, SC * k:SC * (k + 1)]
                        .rearrange("p (s o) -> p s o", o=1).to_broadcast([128, SC, 32]))
                io_b = (iota_bf[:].rearrange("p (o e) -> p o e", o=1)
                        .to_broadcast([128, SC, 32]))
                nc.vector.tensor_tensor(mk[:], io_b, lo_b, op=OP.is_equal)
                nc.vector.tensor_tensor(mk[:], mk[:], gout[:, :, 0:32], op=OP.mult)
                tt_tree(mk, 32, dst_ap, p_mk, "mt")

            # ---- phase 1 stream + rowsum tree; rel gathers interleaved so
            # they drain on the 4 SWDGE queues underneath ----
            s_bf = p_p.tile([128, TILES], bf16)
            rel_emitted = 0
            for j in range(NB):
                r0 = j * 128 * B
                st_t = p_st.tile([128, B, 128], f32, tag="st", name="st_t")
                nc.sync.dma_start(
                    st_t[:],
                    ent[r0:r0 + 128 * B, :].rearrange("(p b) d -> p b d", b=B))
                # level 0: f32+f32 -> bf16 halving, then bf16 tree
                b0 = p_tr.tile([128, B, 64], bf16, tag="b0", name="b0")
                nc.vector.tensor_tensor(b0[:], st_t[:, :, 0:64], st_t[:, :, 64:128],
                                        op=OP.add)
                tt_tree(b0, 64,
                        s_bf[:, B * j:B * (j + 1)]
                        .rearrange("p (b o) -> p b o", o=1),
                        p_tr, "pt")
                if j % 2 == 1 and rel_emitted < NCH:
                    k = rel_emitted
                    sel_chunk("rel", relv, k, rel_val[:, SC * k:SC * (k + 1)])
                    rel_emitted += 1
            while rel_emitted < NCH:
                k = rel_emitted
                sel_chunk("rel", relv, k, rel_val[:, SC * k:SC * (k + 1)])
                rel_emitted += 1

            # ---- phase 1b: x4 block-replicate, natural-order write, gather ----
            # s_bf[p, j*B+b] = S[j*128*B + p*B + b]; table row = j*128 + p,
            # in-row entity c = b (B == 32 entities per row).
            rep_sb = p_p.tile([128, LROWS], bf16)
            nc.vector.tensor_copy(
                rep_sb[:].rearrange("p (j c2 b) -> p j c2 b", j=NB, c2=4, b=B),
                s_bf[:].rearrange("p (j o b) -> p j o b", o=1, b=B)
                .to_broadcast([128, NB, 4, B]))
            nc.sync.dma_start(
                ag_in[:].rearrange("(j p c2 b) -> p j c2 b", j=NB, p=128, c2=4, b=B),
                rep_sb[:].rearrange("p (j c2 b) -> p j c2 b", j=NB, c2=4, b=B))
            if not cfg.get("skip_collective"):
                nc.gpsimd.collective_compute(
                    "AllGather", OP.bypass,
                    replica_groups=[list(range(N_CORES))],
                    ins=[ag_in[:].opt()], outs=[sg[:].opt()])

            # ---- phase 2: src/dst gathers + selects + combine + writeout ----
            for k in range(NCH):
                ssl = slice(SC * k, SC * (k + 1))
                sel_chunk("src", sgv, k, score[:, ssl])
                d_t = p_mk.tile([128, SC], f32, tag="dsel", name="d_t")
                sel_chunk("dst", sgv, k, d_t[:])
                nc.vector.tensor_tensor(score[:, ssl], score[:, ssl],
                                        rel_val[:, ssl], op=OP.add)
                nc.vector.tensor_tensor(score[:, ssl], score[:, ssl], d_t[:],
                                        op=OP.subtract)
                # incremental writeout: flush every completed 32-col group
                g_lo = (SC * k) // 32
                g_hi = (SC * (k + 1)) // 32
                for g in range(g_lo, g_hi):
                    nc.vector.transpose(tr2[:, 32 * g:32 * g + 32],
                                        score[:, 32 * g:32 * g + 32])
                    for a in range(4):
                        nc.scalar.dma_start(ov[a][:, g:g + 1, :],
                                            tr2[32 * a:32 * a + 32,
                                                32 * g:32 * g + 32]
                                            .rearrange("i (g j) -> i g j", g=1))

    nc.finalize()
    return nc


_NC_CACHE = {}


def _get_nc(key, cfg):
    if key not in _NC_CACHE:
        _NC_CACHE[key] = build_nc(cfg)
    return _NC_CACHE[key]


def _prep_idx(raw):
    """raw int array [NEDGE] -> (hi wrapped+replicated int16, lo f32)."""
    raw = np.asarray(raw).astype(np.int64)
    nedge = raw.shape[0]
    hi = (raw >> SHIFT).astype(np.int16)
    lo = (raw & MASK).astype(np.float32)
    hi_w = np.tile(np.ascontiguousarray(hi.reshape(nedge // 16, 16).T), (8, 1))
    lo_t = np.ascontiguousarray(lo.reshape(nedge // 128, 128).T)
    return np.ascontiguousarray(hi_w), lo_t


def shard_inputs(ent_table, rel_table, src_idx, dst_idx, rel_idx, cfg):
    ROWS = cfg["tiles"] * 128
    n_ent = np.asarray(ent_table).shape[0]
    ent = np.ascontiguousarray(np.asarray(ent_table, dtype=np.float32))
    relt = np.ascontiguousarray(np.asarray(rel_table, dtype=np.float32))
    idxs = {"src": np.asarray(src_idx), "dst": np.asarray(dst_idx),
            "rel": np.asarray(rel_idx)}
    in_maps = []
    for c in range(N_CORES):
        lo_r = c * ROWS
        hi_r = min((c + 1) * ROWS, n_ent)
        shard = ent[lo_r:hi_r]
        if hi_r - lo_r < ROWS:
            pad = np.zeros((ROWS - max(hi_r - lo_r, 0), DIM), np.float32)
            shard = np.concatenate([shard, pad], axis=0) if hi_r > lo_r else pad
        m = {"ent_shard": shard, "rel_table": relt}
        for st in ("src", "dst", "rel"):
            hi_w, lo_t = _prep_idx(idxs[st][c])
            m[f"hi_{st}"] = hi_w
            m[f"lo_{st}"] = lo_t
        in_maps.append(m)
    return in_maps


def kernel(ent_table, rel_table, src_idx, dst_idx, rel_idx):
    from concourse.bass_utils import run_bass_kernel_spmd

    cfg = FULL_CFG
    nc = _get_nc("full", cfg)
    in_maps = shard_inputs(ent_table, rel_table, src_idx, dst_idx, rel_idx, cfg)
    res = run_bass_kernel_spmd(nc, in_maps, core_ids=list(range(N_CORES)))
    return np.concatenate([res.results[c]["out"] for c in range(N_CORES)])


# revision 13
# speedup vs baseline: 1.3244x; 1.3244x over previous
"""TransE-style GNN message-passing scoring kernel for 8 Trainium2 NeuronCores.

Math: reference computes scores[r,e] = sum_d(ent[src]+rel[rl]-ent[dst])[d].
The sum over d is linear, so scores = S[src] + R[rl] - S[dst] where
S = rowsum(ent_table) [1M], R = rowsum(rel_table) [1000].

V2 design (per core c = relation row c):
  phase 1: stream 1/8 of ent_table in [128,32,128] super-row tiles (16KB
           contiguous per partition), rowsum via a TT-add tree on DVE
           (2-operand TTs avoid the DVE 2-port perf modes that lock the Q7
           SWDGE cores out of SBUF and stall gather descriptor generation).
           Tree output lands directly in bf16.
  phase 1b: replicate each 32-entry block x4 in SBUF (block-replicated
           bf16 table rows of 256B), one natural-order DMA to DRAM,
           AllGather -> Tg[31744 rows x 128 bf16] = 8.1MB shared table.
           hi = idx>>5 fits int16 (31744 rows).  rel_table rowsums get the
           same 32x128 bf16-x4 local table (8KB, no collective).
  phase 2: dma_gather 256B granules (the SWDGE floor) from Tg / reltab on
           4 SWDGE queues (the ucode max; ~8ns/idx/queue emission is the
           kernel's wall).  rel gathers are issued first so they drain
           under phase 1.  Select lane lo=idx&31 from the 32 bf16
           candidates with an iota/is_equal one-hot, multiply, and a TT-add
           tree (again 2-operand to keep the Q7s unblocked).
  phase 3: score = sel(src) + sel(rel) - sel(dst); 32x32 stream-transposes
           + block-permuted DMA writes out[131072] in edge order; zero tail.
Host does only integer index prep (hi/lo split + SWDGE wrap layout) and
tensor sharding/concat; all FP math runs on device.
"""

import numpy as np

N_ENT = 1_000_000
DIM = 128
R_TYPES = 8
E_PER_TYPE = 131_072
SCORE_DIM = 150_000
N_REL = 1_000
N_CORES = 8

FULL_CFG = dict(
    tiles=992,          # ent rows/128 per core (992*128 = 126,976)
    batch=32,           # stream rows per partition per tile
    e_cols=E_PER_TYPE // 128,   # 1024 score slots ([128, e_cols] per core)
    jch=4096,           # indices per dma_gather instruction
    score_dim=SCORE_DIM,
    queues=4,
)

SHIFT = 5               # table row = idx >> 5 (32 entities per 256B row)
MASK = 31


def build_nc(cfg):
    import concourse.bass as bass
    import concourse.bacc as bacc
    import concourse.tile as tile
    from concourse import mybir

    f32 = mybir.dt.float32
    bf16 = mybir.dt.bfloat16
    i16 = mybir.dt.int16
    AX = mybir.AxisListType
    OP = mybir.AluOpType

    TILES = cfg["tiles"]
    B = cfg["batch"]
    S = cfg["e_cols"]              # score slots (free dim of [128, S])
    JCH = cfg["jch"]
    SCORE = cfg["score_dim"]
    NQ = cfg["queues"]
    ROWS = TILES * 128             # 126,976 ent rows per core
    NB = TILES // B                # stream tiles (31)
    NEDGE = 128 * S                # 131,072 edges per core
    HCOLS = NEDGE // 16            # wrapped-idx columns (8192)
    NCH = NEDGE // JCH             # gather chunks per stream (16)
    SC = JCH // 128                # score slots per chunk (64)
    CHW = JCH // 16                # hi columns per chunk (512)
    NG2 = S // 32                  # 32-col writeout groups (32)
    LROWS = ROWS // 32             # local table rows (3968)
    AGL = LROWS * 128              # local replicated table elems (507,904)
    SGL = N_CORES * AGL            # global table elems (4,063,232)
    GROWS = SGL // 128             # global table rows (31,744)
    assert TILES % B == 0 and NEDGE % JCH == 0 and JCH % 128 == 0
    assert GROWS < 32768
    PAD = SCORE - NEDGE
    assert PAD % 16 == 0

    nc = bacc.Bacc(None, num_devices=N_CORES, num_swdge_queues=NQ)
    ent = nc.dram_tensor("ent_shard", [ROWS, DIM], f32, kind="ExternalInput")
    rel = nc.dram_tensor("rel_table", [N_REL, DIM], f32, kind="ExternalInput")
    his = {}
    los = {}
    for st in ("src", "dst", "rel"):
        his[st] = nc.dram_tensor(f"hi_{st}", [128, HCOLS], i16, kind="ExternalInput")
        los[st] = nc.dram_tensor(f"lo_{st}", [128, S], f32, kind="ExternalInput")
    out = nc.dram_tensor("out", [SCORE], f32, kind="ExternalOutput")
    iota = nc.inline_tensor(
        np.tile(np.arange(32, dtype=np.float32), (128, 1)), name="iota32")
    wtab = nc.inline_tensor(np.zeros((1, 128), dtype=np.float32), name="warmtab")
    widx = nc.inline_tensor(np.zeros((128, 8), dtype=np.int16), name="warmidx")

    with tile.TileContext(nc) as tc:
        with tc.tile_pool(name="stream", bufs=2) as p_st, \
             tc.tile_pool(name="persist", bufs=1) as p_p, \
             tc.tile_pool(name="scratch", bufs=2) as p_scr, \
             tc.tile_pool(name="hi", bufs=6) as p_hi, \
             tc.tile_pool(name="gout", bufs=8) as p_go, \
             tc.tile_pool(name="mask", bufs=3) as p_mk, \
             tc.tile_pool(name="tree", bufs=2) as p_tr, \
             tc.tile_pool(name="dram", bufs=1, space="DRAM") as p_d:

            ag_in = p_d.tile([AGL], bf16)
            sg = p_d.tile([SGL], bf16, addr_space="Shared")
            rg = p_d.tile([1024], f32)
            reltab = p_d.tile([4096], bf16)

            # ---- warm-up gathers: pull the Q7 IRAM library load to t~0 on
            # all 4 SWDGE queues (each queue = its own pair of Q7 cores) ----
            wi_sb = p_p.tile([128, 8], i16)
            nc.sync.dma_start(wi_sb[:], widx[:])
            for q in range(NQ):
                wg = p_p.tile([128, 1, 128], f32, name=f"wg{q}")
                nc.gpsimd.dma_gather(wg[:], wtab[:], wi_sb[:], 128, 128, 128,
                                     single_packet=False, queue_num=q)

            # ---- zero-pad out tail early (independent of all phases) ----
            z = p_p.tile([16, PAD // 16], f32)
            nc.vector.memset(z[:], 0.0)
            nc.sync.dma_start(out[NEDGE:SCORE].rearrange("(p f) -> p f", p=16), z[:])

            # ---- rel-table chain on the scalar HWDGE ring so the sync ring
            # can start streaming ent immediately ----
            r_in = p_scr.tile([125, 8, 128], f32, tag="scr", name="r_in")
            nc.scalar.dma_start(r_in[:], rel[0:1000, :].rearrange("(p b) d -> p b d", b=8))
            rs = p_p.tile([128, 8], f32)
            nc.vector.memset(rs[:], 0.0)
            nc.vector.tensor_reduce(rs[:125, :], r_in[:], axis=AX.X, op=OP.add)
            nc.scalar.dma_start(rg[:].rearrange("(p b) -> p b", b=8), rs[:])
            rel_nat = p_p.tile([32, 32], f32)
            nc.scalar.dma_start(rel_nat[:], rg[:].rearrange("(p c) -> p c", c=32))
            rel_rep = p_p.tile([32, 4, 32], bf16)
            nc.vector.tensor_copy(rel_rep[:],
                                  rel_nat[:].rearrange("p (o c) -> p o c", o=1)
                                  .to_broadcast([32, 4, 32]))
            nc.scalar.dma_start(reltab[:].rearrange("(p f) -> p f", f=128), rel_rep[:])
            relv = reltab[:].rearrange("(n e) -> n e", e=128)    # [32, 128]

            # ---- index metadata loads (scalar ring, needed ~30us in);
            # hi slices stream per-chunk from a small pool in sel_chunk ----
            lo_bf = {}
            for st in ("rel", "src", "dst"):
                lo_f = p_scr.tile([128, S], f32, tag="lo", name=f"lo_{st}_f")
                nc.scalar.dma_start(lo_f[:], los[st][:])
                lo_bf[st] = p_p.tile([128, S], bf16, name=f"lo_{st}_bf")
                nc.vector.tensor_copy(lo_bf[st][:], lo_f[:])
            iota_f = p_p.tile([128, 32], f32)
            nc.sync.dma_start(iota_f[:], iota[:])
            iota_bf = p_p.tile([128, 32], bf16)
            nc.vector.tensor_copy(iota_bf[:], iota_f[:])

            score = p_p.tile([128, S], f32)
            rel_val = p_p.tile([128, S], f32)
            tr2 = p_p.tile([128, S], f32)
            sgv = sg[:].rearrange("(n e) -> n e", e=128)         # [31744, 128]
            ov = out[0:NEDGE].rearrange("(g i a j) -> a i g j", g=NG2, i=32, a=4, j=32)
            qctr = [0]

            def tt_tree(src_ap, width, dst_ap, pool, tag):
                """Reduce innermost `width` lanes by repeated halving TT-adds.

                src_ap: [128, n, width] bf16; dst_ap: [128, n(,1)] f32.
                2-operand TTs only (never triggers DVE 2-port perf modes).
                """
                cur = src_ap
                w = width
                while w > 2:
                    h = w // 2
                    nxt = pool.tile([128, cur.shape[1], h], bf16, tag=f"{tag}{h}",
                                    name=f"{tag}{h}")
                    nc.vector.tensor_tensor(nxt[:], cur[:, :, 0:h], cur[:, :, h:w],
                                            op=OP.add)
                    cur = nxt
                    w = h
                nc.vector.tensor_tensor(dst_ap, cur[:, :, 0:1], cur[:, :, 1:2],
                                        op=OP.add)

            def issue_gather(st, table_v, k):
                """Issue chunk k's hi load + dma_gather; returns the gout tile."""
                hi_t = p_hi.tile([128, CHW], i16, tag="hi", name=f"hi_{st}{k}")
                nc.scalar.dma_start(hi_t[:], his[st][:, CHW * k:CHW * (k + 1)])
                gout = p_go.tile([128, SC, 128], bf16, tag="gout", name="gout")
                nc.gpsimd.dma_gather(
                    gout[:], table_v, hi_t[:], JCH, JCH, 128,
                    single_packet=False, queue_num=qctr[0] % NQ)
                qctr[0] += 1
                return gout

            def do_select(gout, st, k, dst_ap):
                """One-hot select lane lo from gout, write [128, SC] f32."""
                mk = p_mk.tile([128, SC, 32], bf16, tag="mk", name="mk")
                lo_b = (lo_bf[st][:, SC * k:SC * (k + 1)]
                        .rearrange("p (s o) -> p s o", o=1).to_broadcast([128, SC, 32]))
                io_b = (iota_bf[:].rearrange("p (o e) -> p o e", o=1)
                        .to_broadcast([128, SC, 32]))
                nc.vector.tensor_tensor(mk[:], io_b, lo_b, op=OP.is_equal)
                nc.vector.tensor_tensor(mk[:], mk[:], gout[:, :, 0:32], op=OP.mult)
                tt_tree(mk, 32, dst_ap, p_mk, "mt")

            def sel_chunk(st, table_v, k, dst_ap):
                do_select(issue_gather(st, table_v, k), st, k, dst_ap)

            # ---- phase 1 stream + rowsum tree; rel gathers issued up-front
            # so they drain on the 4 SWDGE queues underneath.  Their DVE
            # selects are SKEWED to land only after the gather is done —
            # the DVE FIFO is in-order and a select parked on an unfinished
            # gather would block the stream's tree-reduces behind it. ----
            s_bf = p_p.tile([128, TILES], bf16)
            rel_gouts = {}
            for k in range(min(7, NCH)):
                rel_gouts[k] = issue_gather("rel", relv, k)
            rel_issued = len(rel_gouts)
            rel_sel = 0

            def rel_step():
                """Select the oldest drained rel gout, then issue the next
                rel gather to keep the queues fed."""
                nonlocal rel_issued, rel_sel
                if rel_sel < NCH:
                    k = rel_sel
                    gout = rel_gouts.pop(k)
                    do_select(gout, "rel", k, rel_val[:, SC * k:SC * (k + 1)])
                    rel_sel += 1
                if rel_issued < NCH:
                    k = rel_issued
                    rel_gouts[k] = issue_gather("rel", relv, k)
                    rel_issued += 1

            for j in range(NB):
                r0 = j * 128 * B
                st_t = p_st.tile([128, B, 128], f32, tag="st", name="st_t")
                nc.sync.dma_start(
                    st_t[:],
                    ent[r0:r0 + 128 * B, :].rearrange("(p b) d -> p b d", b=B))
                # level 0: f32+f32 -> bf16 halving, then bf16 tree
                b0 = p_tr.tile([128, B, 64], bf16, tag="b0", name="b0")
                nc.vector.tensor_tensor(b0[:], st_t[:, :, 0:64], st_t[:, :, 64:128],
                                        op=OP.add)
                tt_tree(b0, 64,
                        s_bf[:, B * j:B * (j + 1)]
                        .rearrange("p (b o) -> p b o", o=1),
                        p_tr, "pt")
                if j >= 6:
                    rel_step()
            while rel_sel < NCH:
                rel_step()

            # ---- phase 1b: x4 block-replicate, natural-order write, gather ----
            # s_bf[p, j*B+b] = S[j*128*B + p*B + b]; table row = j*128 + p,
            # in-row entity c = b (B == 32 entities per row).
            rep_sb = p_p.tile([128, LROWS], bf16)
            nc.vector.tensor_copy(
                rep_sb[:].rearrange("p (j c2 b) -> p j c2 b", j=NB, c2=4, b=B),
                s_bf[:].rearrange("p (j o b) -> p j o b", o=1, b=B)
                .to_broadcast([128, NB, 4, B]))
            nc.sync.dma_start(
                ag_in[:].rearrange("(j p c2 b) -> p j c2 b", j=NB, p=128, c2=4, b=B),
                rep_sb[:].rearrange("p (j c2 b) -> p j c2 b", j=NB, c2=4, b=B))
            if not cfg.get("skip_collective"):
                nc.gpsimd.collective_compute(
                    "AllGather", OP.bypass,
                    replica_groups=[list(range(N_CORES))],
                    ins=[ag_in[:].opt()], outs=[sg[:].opt()])

            # ---- phase 2: src/dst gathers + selects + combine + writeout ----
            for k in range(NCH):
                ssl = slice(SC * k, SC * (k + 1))
                sel_chunk("src", sgv, k, score[:, ssl])
                d_t = p_mk.tile([128, SC], f32, tag="dsel", name="d_t")
                sel_chunk("dst", sgv, k, d_t[:])
                nc.vector.tensor_tensor(score[:, ssl], score[:, ssl],
                                        rel_val[:, ssl], op=OP.add)
                nc.vector.tensor_tensor(score[:, ssl], score[:, ssl], d_t[:],
                                        op=OP.subtract)
                # incremental writeout: flush every completed 32-col group
                g_lo = (SC * k) // 32
                g_hi = (SC * (k + 1)) // 32
                for g in range(g_lo, g_hi):
                    nc.vector.transpose(tr2[:, 32 * g:32 * g + 32],
                                        score[:, 32 * g:32 * g + 32])
                    for a in range(4):
                        nc.scalar.dma_start(ov[a][:, g:g + 1, :],
                                            tr2[32 * a:32 * a + 32,
                                                32 * g:32 * g + 32]
                                            .rearrange("i (g j) -> i g j", g=1))

    nc.finalize()
    return nc


_NC_CACHE = {}


def _get_nc(key, cfg):
    if key not in _NC_CACHE:
        _NC_CACHE[key] = build_nc(cfg)
    return _NC_CACHE[key]


def _prep_idx(raw):
    """raw int array [NEDGE] -> (hi wrapped+replicated int16, lo f32)."""
    raw = np.asarray(raw).astype(np.int64)
    nedge = raw.shape[0]
    hi = (raw >> SHIFT).astype(np.int16)
    lo = (raw & MASK).astype(np.float32)
    hi_w = np.tile(np.ascontiguousarray(hi.reshape(nedge // 16, 16).T), (8, 1))
    lo_t = np.ascontiguousarray(lo.reshape(nedge // 128, 128).T)
    return np.ascontiguousarray(hi_w), lo_t


def shard_inputs(ent_table, rel_table, src_idx, dst_idx, rel_idx, cfg):
    ROWS = cfg["tiles"] * 128
    n_ent = np.asarray(ent_table).shape[0]
    ent = np.ascontiguousarray(np.asarray(ent_table, dtype=np.float32))
    relt = np.ascontiguousarray(np.asarray(rel_table, dtype=np.float32))
    idxs = {"src": np.asarray(src_idx), "dst": np.asarray(dst_idx),
            "rel": np.asarray(rel_idx)}
    in_maps = []
    for c in range(N_CORES):
        lo_r = c * ROWS
        hi_r = min((c + 1) * ROWS, n_ent)
        shard = ent[lo_r:hi_r]
        if hi_r - lo_r < ROWS:
            pad = np.zeros((ROWS - max(hi_r - lo_r, 0), DIM), np.float32)
            shard = np.concatenate([shard, pad], axis=0) if hi_r > lo_r else pad
        m = {"ent_shard": shard, "rel_table": relt}
        for st in ("src", "dst", "rel"):
            hi_w, lo_t = _prep_idx(idxs[st][c])
            m[f"hi_{st}"] = hi_w
            m[f"lo_{st}"] = lo_t
        in_maps.append(m)
    return in_maps


def kernel(ent_table, rel_table, src_idx, dst_idx, rel_idx):
    from concourse.bass_utils import run_bass_kernel_spmd

    cfg = FULL_CFG
    nc = _get_nc("full", cfg)
    in_maps = shard_inputs(ent_table, rel_table, src_idx, dst_idx, rel_idx, cfg)
    res = run_bass_kernel_spmd(nc, in_maps, core_ids=list(range(N_CORES)))
    return np.concatenate([res.results[c]["out"] for c in range(N_CORES)])
